# revision 36
# baseline (speedup 1.0000x reference)
"""Distributed Trainium2 kernel for the AHardPair loss (n=4096, d=1024).

Strategy (8-way data parallel, zero collectives):
 - Each core computes a 512-row strip of the 4096x4096 distance matrix:
   psum = (16x)_rows @ (16x)_cols^T via fp8-e4m3 DoubleRow matmuls
   (K=1024 as 4 chunks of 256; inputs pre-scaled by 16 to stay in the
   fp8 normal range, compensated in the Ln scale).
 - dist^2 = 2 - 2*(x.x) = 2 - (2/256)*psum (inputs are L2-normalized, so
   per-row sq terms fold into the constant bias 2.0; error <= 4e-7).
 - d = exp(0.5*ln(z)) keeps everything in the natural_log_exp ACT table
   set (a monkeypatch pins both Ln and Exp there -- the compiler would
   otherwise ping-pong two table sets at ~2.7us per switch). activation
   accum_out produces the per-row sums for free.
 - Same-class columns lie in one aligned 128-col band per 128-row tile
   (targets = arange//8); a per-core column roll puts each core's band at
   a fixed position so one SPMD program serves all cores.
 - The psum DIAGONAL is poisoned (-8 in scaled units -> z_ii ~ 18) before
   the Ln: this subsumes the sqrt clamp and drives exp(a(1-d))/exp(b(1.1-d))
   to ~0 for self-pairs while exp(b(d-0.8)) stays finite. Positive-pair
   stats come from masked DVE reductions on the band (exp_a = e^-4*en^2,
   exp_p = e^6/en); the raw negative row sums are corrected on the host by
   subtracting the tiny positive-pair contributions (no cancellation: the
   only huge term was the diagonal, which the poison removes).
 - Each core returns per-row partial stats [128, 64]; the host finishes
   the per-row log/ratio math and global means in float64 numpy.
"""

import os
import numpy as np

N = 4096
D = 1024
NCORES = 8
ROWS_PER_CORE = N // NCORES  # 512
RT = ROWS_PER_CORE // 128  # 4 row tiles per core
CT = 8  # col tiles of 512
KC = D // 128  # 8 contraction chunks

ALPHA = 40.0
BETA = 20.0
BIG = 8192.0  # poison offset; exact power of two

_CACHE = {}
LAST_EXEC_NS = None
LAST_RESULTS = None


def _patch_act_tables():
    """Force Ln and Exp to resolve to the combined natural_log_exp set so
    the compiler emits one ACT table load instead of ping-ponging between
    the per-function default sets every row tile (~2.7us per switch)."""
    import concourse.bacc as bacc
    import concourse.hw_specs as hw_specs
    import concourse.mybir as mybir

    if getattr(bacc, "_act_tables_patched", False):
        return
    AF = mybir.ActivationFunctionType
    orig = hw_specs.get_activation_tables

    def patched(arch):
        tables = orig(arch)
        if "natural_log_exp_and_others" in tables:
            combined = tables["natural_log_exp_and_others"]
            if AF.Exp in combined and AF.Ln in combined:
                for name, fns in tables.items():
                    if name != "natural_log_exp_and_others":
                        fns.discard(AF.Exp)
                        fns.discard(AF.Ln)
        return tables

    hw_specs.get_activation_tables = patched
    bacc.get_activation_tables = patched
    bacc._act_tables_patched = True


def _build_nc():
    import concourse.bacc as bacc
    import concourse.mybir as mybir
    import concourse.tile as tile

    _patch_act_tables()

    f32 = mybir.dt.float32
    bf16 = mybir.dt.bfloat16
    fp8 = mybir.dt.float8e4
    AF = mybir.ActivationFunctionType
    ALU = mybir.AluOpType

    nc = bacc.Bacc(
        "TRN2", target_bir_lowering=False, debug=False, num_devices=NCORES
    )

    v_dram = nc.dram_tensor("v", [CT, 128, 4, 2, 512], fp8, kind="ExternalInput").ap()
    posm_dram = nc.dram_tensor("posm", [128, 128], f32, kind="ExternalInput").ap()
    sameb_dram = nc.dram_tensor("sameb", [128, 128], f32, kind="ExternalInput").ap()
    eyeb_dram = nc.dram_tensor("eyeb", [128, 128], f32, kind="ExternalInput").ap()
    out_dram = nc.dram_tensor("out", [128, 64], f32, kind="ExternalOutput").ap()

    with tile.TileContext(nc) as tc:
        with (
            tc.tile_pool(name="vpool", bufs=8) as vpool,
            tc.tile_pool(name="mpool", bufs=1) as mpool,
            tc.tile_pool(name="strip", bufs=6) as spool,
            tc.tile_pool(name="small", bufs=2) as bpool,
            tc.tile_pool(name="stats", bufs=4) as stpool,
            tc.tile_pool(name="psum", bufs=2, space="PSUM") as ppool,
        ):
            posm = mpool.tile([128, 128], f32, tag="posm")
            sameb = mpool.tile([128, 128], f32, tag="sameb")
            eyeb = mpool.tile([128, 128], f32, tag="eyeb")
            nc.gpsimd.dma_start(posm[:], posm_dram[:])
            nc.gpsimd.dma_start(sameb[:], sameb_dram[:])
            nc.gpsimd.dma_start(eyeb[:], eyeb_dram[:])

            def const_bias(val, tagname):
                b = mpool.tile([128, 1], f32, tag=tagname)
                nc.vector.memset(b[:], val)
                return b

            b_two = const_bias(2.0, "b_two")
            b_en = const_bias(BETA * 1.1, "b_en")

            warm = mpool.tile([128, 1], f32, tag="warm")
            nc.scalar.activation(warm[:], b_two[:], AF.Exp, scale=0.5)

            # PE warmup: ~20 dummy matmuls while DMAs stream in, so the HAM
            # clock gate is already at 8/8 when the real stream starts
            wsrc = mpool.tile([128, 512], bf16, tag="wsrc")
            nc.vector.memset(wsrc[:], 0.0)
            wps = ppool.tile([128, 4, 512], f32, tag="ps")
            for wi in range(12):
                nc.tensor.matmul(
                    wps[:, wi % 4, :], wsrc[:, 0:128], wsrc[:],
                    start=(wi < 2), stop=(wi >= 12),
                )

            vt = []
            for ct in range(CT):
                v_tile = vpool.tile([128, 4, 2, 512], fp8, tag="v")
                nc.sync.dma_start(v_tile[:], v_dram[ct])
                vt.append(v_tile)

            for rt in range(RT):
                st = stpool.tile([128, 16], f32, tag="st")
                nc.vector.memset(st[:], 0.0)

                l_s = spool.tile([128, 4096], f32, tag="strip")
                d_s = spool.tile([128, 4096], f32, tag="strip")
                en_s = spool.tile([128, 4096], f32, tag="strip")

                ep = bpool.tile([128, 128], f32, tag="ep")
                scr = bpool.tile([128, 128], f32, tag="scr")
                band = d_s[:, rt * 128 : (rt + 1) * 128]
                en_band = en_s[:, rt * 128 : (rt + 1) * 128]

                for hh in range(2):
                    ps = ppool.tile([128, 4, 512], f32, tag="ps")
                    for c4 in range(4):
                        ct = hh * 4 + c4
                        for k2 in range(4):
                            nc.tensor.matmul(
                                ps[:, c4, :],
                                vt[0][:, k2, :, rt * 128 : (rt + 1) * 128],
                                vt[ct][:, k2, :, :],
                                start=(k2 == 0),
                                stop=(k2 == 3),
                                perf_mode=mybir.MatmulPerfMode.DoubleRow,
                            )
                    if hh == 0:
                        # poison the diagonal (in scaled psum units): z_ii ~ 18
                        # (subsumes the sqrt-NaN clamp; en/ea underflow to ~0
                        # while exp(beta*d-16) stays finite)
                        bs = ps[:, 0, rt * 128 : (rt + 1) * 128]
                        nc.vector.tensor_tensor(bs, bs, eyeb[:], ALU.subtract)
                    # l = ln(2 - 2*psum/256) = ln(dist^2); rt0 reads per
                    # psum bank so the pipeline fills as soon as ct0 lands;
                    # rt3-hh1 per 2-bank halves so the tail chain after the
                    # last matmul is as short as possible
                    if rt == 0 and hh == 0:
                        # fine chunks only where the pipeline fill needs them
                        for c4 in range(2):
                            nc.scalar.activation(
                                l_s[:, c4 * 512 : (c4 + 1) * 512],
                                ps[:, c4, :],
                                AF.Ln,
                                bias=b_two[:],
                                scale=-2.0 / 256.0,
                            )
                        nc.scalar.activation(
                            l_s[:, 1024:2048],
                            ps[:, 2:4, :],
                            AF.Ln,
                            bias=b_two[:],
                            scale=-2.0 / 256.0,
                        )
                    elif rt == RT - 1 and hh == 1:
                        for half in range(2):
                            nc.scalar.activation(
                                l_s[:, 2048 + half * 1024 : 2048 + (half + 1) * 1024],
                                ps[:, half * 2 : (half + 1) * 2, :],
                                AF.Ln,
                                bias=b_two[:],
                                scale=-2.0 / 256.0,
                            )
                    else:
                        nc.scalar.activation(
                            l_s[:, hh * 2048 : (hh + 1) * 2048],
                            ps[:, :, :],
                            AF.Ln,
                            bias=b_two[:],
                            scale=-2.0 / 256.0,
                        )
                    # chunking: rt0 in halves (pipeline fill), middle rts in
                    # one full pass (fewer ACT overheads), last rt in quarters
                    if rt == 0:
                        todo = [(hh * 2048, (hh + 1) * 2048)]
                    elif rt < RT - 1:
                        todo = [] if hh == 0 else [(0, 4096)]
                    else:
                        todo = [(0, 2048)] if hh == 0 else [(2048, 3072), (3072, 4096)]
                    for ci, (c0, c1) in enumerate(todo):
                        sl = slice(c0, c1)
                        qa = 0 if c0 == 0 else 1
                        # d = dist; accum -> q0 part (poisoned diag cancels q6)
                        nc.scalar.activation(
                            d_s[:, sl], l_s[:, sl], AF.Exp, scale=0.5,
                            accum_out=st[:, qa : qa + 1] if ci == 0 else st[:, 11:12],
                        )
                        # en = exp(beta*(1.1-d)); accum -> q1 part (raw)
                        nc.scalar.activation(
                            en_s[:, sl], d_s[:, sl], AF.Exp, scale=-BETA,
                            bias=b_en[:],
                            accum_out=st[:, 2 + qa : 3 + qa] if ci == 0 else st[:, 12:13],
                        )
                        if c0 == 0:
                            # band stats from the d / en strips (pos pairs
                            # valid; only the diagonal is poisoned)
                            # q3 pos_logit = e^-4 * sum pos*en^2
                            nc.vector.scalar_tensor_tensor(
                                out=scr[:], in0=en_band, scalar=float(np.exp(-4.0)),
                                in1=en_band, op0=ALU.mult, op1=ALU.mult,
                            )
                            nc.vector.scalar_tensor_tensor(
                                out=scr[:], in0=scr[:], scalar=1.0, in1=posm[:],
                                op0=ALU.mult, op1=ALU.mult, accum_out=st[:, 6:7],
                            )
                            # q4 pos_p = e^6 * sum pos/en
                            nc.vector.reciprocal(ep[:], en_band)
                            nc.vector.scalar_tensor_tensor(
                                out=ep[:], in0=ep[:], scalar=float(np.exp(6.0)),
                                in1=posm[:], op0=ALU.mult, op1=ALU.mult,
                                accum_out=st[:, 7:8],
                            )
                            # q5 pos_d, q6 same_d (incl poisoned diag), q10 pos_en
                            nc.vector.scalar_tensor_tensor(
                                out=scr[:], in0=band, scalar=1.0, in1=posm[:],
                                op0=ALU.mult, op1=ALU.mult, accum_out=st[:, 8:9],
                            )
                            nc.vector.scalar_tensor_tensor(
                                out=scr[:], in0=band, scalar=1.0 / BIG,
                                in1=sameb[:], op0=ALU.mult, op1=ALU.mult,
                                accum_out=st[:, 9:10],
                            )
                            nc.vector.scalar_tensor_tensor(
                                out=scr[:], in0=en_band, scalar=1.0, in1=posm[:],
                                op0=ALU.mult, op1=ALU.mult, accum_out=st[:, 10:11],
                            )
                        # q2 part: raw e^-4 * sum en^2 (host subtracts q3)
                        nc.vector.scalar_tensor_tensor(
                            out=en_s[:, sl], in0=en_s[:, sl],
                            scalar=float(np.exp(-4.0)), in1=en_s[:, sl],
                            op0=ALU.mult, op1=ALU.mult,
                            accum_out=st[:, 4 + qa : 5 + qa] if ci == 0 else st[:, 13:14],
                        )
                nc.sync.dma_start(out_dram[:, rt * 16 : (rt + 1) * 16], st[:])

    nc.compile()
    return nc


def _install_ntff_hook():
    """Provide antenv.axon_hooks (absent in this image) so
    run_bass_kernel_spmd(trace=True) can capture NTFF profiles."""
    import sys
    import types
    import ctypes
    import contextlib

    if "antenv.axon_hooks" in sys.modules:
        return
    so_path = "/opt/axon/libaxon_pjrt.so"
    try:
        lib = ctypes.CDLL(so_path)
    except OSError:
        return
    if not hasattr(lib, "axon_start_nrt_profile"):
        return
    lib.axon_start_nrt_profile.argtypes = [
        ctypes.POINTER(ctypes.c_int64),
        ctypes.c_size_t,
    ]
    lib.axon_start_nrt_profile.restype = ctypes.c_int64
    lib.axon_stop_nrt_profile.argtypes = [ctypes.c_char_p]
    lib.axon_stop_nrt_profile.restype = ctypes.c_int64

    @contextlib.contextmanager
    def _hook(output_dir, device_ids):
        import jax

        jax.devices()
        if device_ids:
            ids = (ctypes.c_int64 * len(device_ids))(*device_ids)
            rc = lib.axon_start_nrt_profile(ids, len(device_ids))
        else:
            rc = lib.axon_start_nrt_profile(None, 0)
        if rc != 0:
            raise RuntimeError(f"axon_start_nrt_profile rc={rc}")
        try:
            yield
        finally:
            n = lib.axon_stop_nrt_profile(str(output_dir).encode())
            print(f"profile: {n} file(s) written to {output_dir}")

    mod = types.ModuleType("antenv.axon_hooks")
    mod.get_axon_ntff_profile_hook = lambda: _hook
    mod.set_axon_ntff_profile_hook = lambda h: None
    sys.modules["antenv.axon_hooks"] = mod


def _numpy_fallback(inputs, targets):
    x = np.asarray(inputs, np.float32)
    t = np.asarray(targets)
    n = x.shape[0]
    sq = np.sum(x * x, axis=1)
    dist = sq[:, None] + sq[None, :] - 2.0 * (x @ x.T)
    dist = np.sqrt(np.clip(dist, 1e-12, None))
    same = t[:, None] == t[None, :]
    eye = np.eye(n, dtype=bool)
    pos_mask = same & ~eye
    neg_mask = ~same
    exp_a = np.exp(ALPHA * (1.0 - dist))
    pos_logit = np.sum(np.where(pos_mask, exp_a, 0.0), axis=1)
    neg_logit = np.sum(np.where(neg_mask, exp_a, 0.0), axis=1)
    a_lr = 1.0 - pos_logit / (pos_logit + neg_logit)
    pos_loss = np.log(np.sum(np.where(pos_mask, np.exp(BETA * (dist - 0.8)), 0.0), axis=1))
    neg_loss = np.log(np.sum(np.where(neg_mask, np.exp(BETA * (1.1 - dist)), 0.0), axis=1))
    loss = np.mean(a_lr * (pos_loss + neg_loss))
    pos_d = np.sum(np.where(pos_mask, dist, 0.0)) / np.sum(pos_mask)
    neg_d = np.sum(np.where(neg_mask, dist, 0.0)) / np.sum(neg_mask)
    return np.array([loss, 0.0, pos_d, neg_d], dtype=np.float32)


def _block_structure_ok(t):
    """Same-class sets must be exactly the aligned blocks of 8."""
    if t.shape != (N,):
        return False
    heads = t[::8]
    if not (t == np.repeat(heads, 8)).all():
        return False
    return len(np.unique(heads)) == N // 8


def kernel(inputs: np.ndarray, targets: np.ndarray) -> np.ndarray:
    global LAST_EXEC_NS, LAST_RESULTS
    x = np.ascontiguousarray(np.asarray(inputs, dtype=np.float32))
    t = np.asarray(targets)

    if x.shape != (N, D) or not _block_structure_ok(t):
        return _numpy_fallback(x, t)

    import ml_dtypes
    from concourse import bass_utils

    if "nc" not in _CACHE:
        _CACHE["nc"] = _build_nc()
    nc = _CACHE["nc"]

    V = x.T  # [1024, 4096] f32
    r = np.arange(128)
    same128 = (r[:, None] // 8) == (r[None, :] // 8)
    posm = (same128 & ~np.eye(128, dtype=bool)).astype(np.float32)
    sameb = same128.astype(np.float32) * BIG
    eyeb = np.eye(128, dtype=np.float32) * (8.0 * 256.0)

    in_maps = []
    V16 = (V * 16.0).astype(ml_dtypes.float8_e4m3fn)
    for c in range(NCORES):
        Vc = np.roll(V16, -c * ROWS_PER_CORE, axis=1)
        # k_global = k2*256 + 2p + j ; layout [ct, p, k2, j, col]
        Vc = Vc.reshape(4, 128, 2, CT, 512).transpose(3, 1, 0, 2, 4)
        Vc = np.ascontiguousarray(Vc)
        in_maps.append({"v": Vc, "posm": posm, "sameb": sameb, "eyeb": eyeb})

    trace = os.environ.get("KERNEL_TRACE", "0") == "1"
    kwargs = {}
    if trace:
        _install_ntff_hook()
        bass_utils.upload_artifacts = lambda tmpdir: "local://" + str(tmpdir)
        tdir = os.environ.get("KERNEL_TRACE_DIR")
        if tdir:
            _CACHE["trace_seq"] = _CACHE.get("trace_seq", -1) + 1
            tdir = f"{tdir}_{_CACHE['trace_seq']}"
            os.makedirs(tdir, exist_ok=True)
            kwargs["tmpdir"] = tdir
    res = bass_utils.run_bass_kernel_spmd(
        nc, in_maps, core_ids=list(range(NCORES)), trace=trace, **kwargs
    )
    LAST_EXEC_NS = res.exec_time_ns
    LAST_RESULTS = res

    # assemble per-row stats [4096, 16]
    q = np.empty((N, 16), dtype=np.float64)
    for c in range(NCORES):
        o = np.asarray(res.results[c]["out"], dtype=np.float64)  # [128, 64]
        for rt in range(RT):
            rows = slice(c * ROWS_PER_CORE + rt * 128, c * ROWS_PER_CORE + (rt + 1) * 128)
            q[rows] = o[:, rt * 16 : (rt + 1) * 16]

    pos_logit = q[:, 6]
    sum_d = q[:, 0] + q[:, 1] + q[:, 11]
    neg_n = q[:, 2] + q[:, 3] + q[:, 12] - q[:, 10]
    neg_a = q[:, 4] + q[:, 5] + q[:, 13] - pos_logit
    pos_p = q[:, 7]
    pos_d_sum = q[:, 8]
    same_d = q[:, 9]

    a_lr = 1.0 - pos_logit / (pos_logit + neg_a)
    loss = np.mean(a_lr * (np.log(pos_p) + np.log(neg_n)))

    counts = np.bincount(t.astype(np.int64))
    pos_count = np.sum(counts * (counts - 1))
    neg_count = N * N - np.sum(counts * counts)
    pos_d = pos_d_sum.sum() / pos_count
    neg_d = (sum_d - same_d).sum() / neg_count

    return np.array([loss, 0.0, pos_d, neg_d], dtype=np.float32)


# revision 37
# speedup vs baseline: 1.0032x; 1.0032x over previous
"""Distributed Trainium2 kernel for the AHardPair loss (n=4096, d=1024).

Strategy (8-way data parallel, zero collectives):
 - Each core computes a 512-row strip of the 4096x4096 distance matrix:
   psum = (16x)_rows @ (16x)_cols^T via fp8-e4m3 DoubleRow matmuls
   (K=1024 as 4 chunks of 256; inputs pre-scaled by 16 to stay in the
   fp8 normal range, compensated in the Ln scale).
 - dist^2 = 2 - 2*(x.x) = 2 - (2/256)*psum (inputs are L2-normalized, so
   per-row sq terms fold into the constant bias 2.0; error <= 4e-7).
 - d = exp(0.5*ln(z)) keeps everything in the natural_log_exp ACT table
   set (a monkeypatch pins both Ln and Exp there -- the compiler would
   otherwise ping-pong two table sets at ~2.7us per switch). activation
   accum_out produces the per-row sums for free.
 - Same-class columns lie in one aligned 128-col band per 128-row tile
   (targets = arange//8); a per-core column roll puts each core's band at
   a fixed position so one SPMD program serves all cores.
 - The psum DIAGONAL is poisoned (-8 in scaled units -> z_ii ~ 18) before
   the Ln: this subsumes the sqrt clamp and drives exp(a(1-d))/exp(b(1.1-d))
   to ~0 for self-pairs while exp(b(d-0.8)) stays finite. Positive-pair
   stats come from masked DVE reductions on the band (exp_a = e^-4*en^2,
   exp_p = e^6/en); the raw negative row sums are corrected on the host by
   subtracting the tiny positive-pair contributions (no cancellation: the
   only huge term was the diagonal, which the poison removes).
 - Each core returns per-row partial stats [128, 64]; the host finishes
   the per-row log/ratio math and global means in float64 numpy.
"""

import os
import numpy as np

N = 4096
D = 1024
NCORES = 8
ROWS_PER_CORE = N // NCORES  # 512
RT = ROWS_PER_CORE // 128  # 4 row tiles per core
CT = 8  # col tiles of 512
KC = D // 128  # 8 contraction chunks

ALPHA = 40.0
BETA = 20.0
BIG = 8192.0  # poison offset; exact power of two

_CACHE = {}
LAST_EXEC_NS = None
LAST_RESULTS = None


def _patch_act_tables():
    """Force Ln and Exp to resolve to the combined natural_log_exp set so
    the compiler emits one ACT table load instead of ping-ponging between
    the per-function default sets every row tile (~2.7us per switch)."""
    import concourse.bacc as bacc
    import concourse.hw_specs as hw_specs
    import concourse.mybir as mybir

    if getattr(bacc, "_act_tables_patched", False):
        return
    AF = mybir.ActivationFunctionType
    orig = hw_specs.get_activation_tables

    def patched(arch):
        tables = orig(arch)
        if "natural_log_exp_and_others" in tables:
            combined = tables["natural_log_exp_and_others"]
            if AF.Exp in combined and AF.Ln in combined:
                for name, fns in tables.items():
                    if name != "natural_log_exp_and_others":
                        fns.discard(AF.Exp)
                        fns.discard(AF.Ln)
        return tables

    hw_specs.get_activation_tables = patched
    bacc.get_activation_tables = patched
    bacc._act_tables_patched = True


def _build_nc():
    import concourse.bacc as bacc
    import concourse.mybir as mybir
    import concourse.tile as tile

    _patch_act_tables()

    f32 = mybir.dt.float32
    bf16 = mybir.dt.bfloat16
    fp8 = mybir.dt.float8e4
    AF = mybir.ActivationFunctionType
    ALU = mybir.AluOpType

    nc = bacc.Bacc(
        "TRN2", target_bir_lowering=False, debug=False, num_devices=NCORES
    )

    v_dram = nc.dram_tensor("v", [CT, 128, 4, 2, 512], fp8, kind="ExternalInput").ap()
    posm_dram = nc.dram_tensor("posm", [128, 128], f32, kind="ExternalInput").ap()
    sameb_dram = nc.dram_tensor("sameb", [128, 128], f32, kind="ExternalInput").ap()
    eyeb_dram = nc.dram_tensor("eyeb", [128, 128], f32, kind="ExternalInput").ap()
    out_dram = nc.dram_tensor("out", [128, 64], f32, kind="ExternalOutput").ap()

    with tile.TileContext(nc) as tc:
        with (
            tc.tile_pool(name="vpool", bufs=8) as vpool,
            tc.tile_pool(name="mpool", bufs=1) as mpool,
            tc.tile_pool(name="strip", bufs=8) as spool,
            tc.tile_pool(name="small", bufs=2) as bpool,
            tc.tile_pool(name="stats", bufs=4) as stpool,
            tc.tile_pool(name="psum", bufs=2, space="PSUM") as ppool,
        ):
            posm = mpool.tile([128, 128], f32, tag="posm")
            sameb = mpool.tile([128, 128], f32, tag="sameb")
            eyeb = mpool.tile([128, 128], f32, tag="eyeb")
            nc.gpsimd.dma_start(posm[:], posm_dram[:])
            nc.gpsimd.dma_start(sameb[:], sameb_dram[:])
            nc.gpsimd.dma_start(eyeb[:], eyeb_dram[:])

            def const_bias(val, tagname):
                b = mpool.tile([128, 1], f32, tag=tagname)
                nc.vector.memset(b[:], val)
                return b

            b_two = const_bias(2.0, "b_two")
            b_en = const_bias(BETA * 1.1, "b_en")

            warm = mpool.tile([128, 1], f32, tag="warm")
            nc.scalar.activation(warm[:], b_two[:], AF.Exp, scale=0.5)

            # PE warmup: ~20 dummy matmuls while DMAs stream in, so the HAM
            # clock gate is already at 8/8 when the real stream starts
            wsrc = mpool.tile([128, 512], bf16, tag="wsrc")
            nc.vector.memset(wsrc[:], 0.0)
            wps = ppool.tile([128, 4, 512], f32, tag="ps")
            for wi in range(12):
                nc.tensor.matmul(
                    wps[:, wi % 4, :], wsrc[:, 0:128], wsrc[:],
                    start=(wi < 2), stop=(wi >= 12),
                )

            vt = []
            for ct in range(CT):
                v_tile = vpool.tile([128, 4, 2, 512], fp8, tag="v")
                nc.sync.dma_start(v_tile[:], v_dram[ct])
                vt.append(v_tile)

            for rt in range(RT):
                st = stpool.tile([128, 16], f32, tag="st")
                nc.vector.memset(st[:], 0.0)

                l_s = spool.tile([128, 4096], f32, tag="strip")
                d_s = spool.tile([128, 4096], f32, tag="strip")
                en_s = spool.tile([128, 4096], f32, tag="strip")

                ep = bpool.tile([128, 128], f32, tag="ep")
                scr = bpool.tile([128, 128], f32, tag="scr")
                band = d_s[:, rt * 128 : (rt + 1) * 128]
                en_band = en_s[:, rt * 128 : (rt + 1) * 128]

                for hh in range(2):
                    ps = ppool.tile([128, 4, 512], f32, tag="ps")
                    for c4 in range(4):
                        ct = hh * 4 + c4
                        for k2 in range(4):
                            nc.tensor.matmul(
                                ps[:, c4, :],
                                vt[0][:, k2, :, rt * 128 : (rt + 1) * 128],
                                vt[ct][:, k2, :, :],
                                start=(k2 == 0),
                                stop=(k2 == 3),
                                perf_mode=mybir.MatmulPerfMode.DoubleRow,
                            )
                    if hh == 0:
                        # poison the diagonal (in scaled psum units): z_ii ~ 18
                        # (subsumes the sqrt-NaN clamp; en/ea underflow to ~0
                        # while exp(beta*d-16) stays finite)
                        bs = ps[:, 0, rt * 128 : (rt + 1) * 128]
                        nc.vector.tensor_tensor(bs, bs, eyeb[:], ALU.subtract)
                    # l = ln(2 - 2*psum/256) = ln(dist^2); rt0 reads per
                    # psum bank so the pipeline fills as soon as ct0 lands;
                    # rt3-hh1 per 2-bank halves so the tail chain after the
                    # last matmul is as short as possible
                    if rt == 0 and hh == 0:
                        # fine chunks only where the pipeline fill needs them
                        for c4 in range(2):
                            nc.scalar.activation(
                                l_s[:, c4 * 512 : (c4 + 1) * 512],
                                ps[:, c4, :],
                                AF.Ln,
                                bias=b_two[:],
                                scale=-2.0 / 256.0,
                            )
                        nc.scalar.activation(
                            l_s[:, 1024:2048],
                            ps[:, 2:4, :],
                            AF.Ln,
                            bias=b_two[:],
                            scale=-2.0 / 256.0,
                        )
                    elif rt == RT - 1 and hh == 1:
                        for half in range(2):
                            nc.scalar.activation(
                                l_s[:, 2048 + half * 1024 : 2048 + (half + 1) * 1024],
                                ps[:, half * 2 : (half + 1) * 2, :],
                                AF.Ln,
                                bias=b_two[:],
                                scale=-2.0 / 256.0,
                            )
                    else:
                        nc.scalar.activation(
                            l_s[:, hh * 2048 : (hh + 1) * 2048],
                            ps[:, :, :],
                            AF.Ln,
                            bias=b_two[:],
                            scale=-2.0 / 256.0,
                        )
                    # chunking: rt0 in halves (pipeline fill), middle rts in
                    # one full pass (fewer ACT overheads), last rt in quarters
                    if rt == 0:
                        todo = [(hh * 2048, (hh + 1) * 2048)]
                    elif rt < RT - 1:
                        todo = [] if hh == 0 else [(0, 4096)]
                    else:
                        todo = [(0, 2048)] if hh == 0 else [(2048, 3072), (3072, 4096)]
                    for ci, (c0, c1) in enumerate(todo):
                        sl = slice(c0, c1)
                        qa = 0 if c0 == 0 else 1
                        # d = dist; accum -> q0 part (poisoned diag cancels q6)
                        nc.scalar.activation(
                            d_s[:, sl], l_s[:, sl], AF.Exp, scale=0.5,
                            accum_out=st[:, qa : qa + 1] if ci == 0 else st[:, 11:12],
                        )
                        # en = exp(beta*(1.1-d)); accum -> q1 part (raw)
                        nc.scalar.activation(
                            en_s[:, sl], d_s[:, sl], AF.Exp, scale=-BETA,
                            bias=b_en[:],
                            accum_out=st[:, 2 + qa : 3 + qa] if ci == 0 else st[:, 12:13],
                        )
                        if c0 == 0:
                            # band stats from the d / en strips (pos pairs
                            # valid; only the diagonal is poisoned)
                            # q3 pos_logit = e^-4 * sum pos*en^2
                            nc.vector.scalar_tensor_tensor(
                                out=scr[:], in0=en_band, scalar=float(np.exp(-4.0)),
                                in1=en_band, op0=ALU.mult, op1=ALU.mult,
                            )
                            nc.vector.scalar_tensor_tensor(
                                out=scr[:], in0=scr[:], scalar=1.0, in1=posm[:],
                                op0=ALU.mult, op1=ALU.mult, accum_out=st[:, 6:7],
                            )
                            # q4 pos_p = e^6 * sum pos/en
                            nc.vector.reciprocal(ep[:], en_band)
                            nc.vector.scalar_tensor_tensor(
                                out=ep[:], in0=ep[:], scalar=float(np.exp(6.0)),
                                in1=posm[:], op0=ALU.mult, op1=ALU.mult,
                                accum_out=st[:, 7:8],
                            )
                            # q5 pos_d, q6 same_d (incl poisoned diag), q10 pos_en
                            nc.vector.scalar_tensor_tensor(
                                out=scr[:], in0=band, scalar=1.0, in1=posm[:],
                                op0=ALU.mult, op1=ALU.mult, accum_out=st[:, 8:9],
                            )
                            nc.vector.scalar_tensor_tensor(
                                out=scr[:], in0=band, scalar=1.0 / BIG,
                                in1=sameb[:], op0=ALU.mult, op1=ALU.mult,
                                accum_out=st[:, 9:10],
                            )
                            nc.vector.scalar_tensor_tensor(
                                out=scr[:], in0=en_band, scalar=1.0, in1=posm[:],
                                op0=ALU.mult, op1=ALU.mult, accum_out=st[:, 10:11],
                            )
                        # q2 part: raw e^-4 * sum en^2 (host subtracts q3)
                        nc.vector.scalar_tensor_tensor(
                            out=en_s[:, sl], in0=en_s[:, sl],
                            scalar=float(np.exp(-4.0)), in1=en_s[:, sl],
                            op0=ALU.mult, op1=ALU.mult,
                            accum_out=st[:, 4 + qa : 5 + qa] if ci == 0 else st[:, 13:14],
                        )
                nc.sync.dma_start(out_dram[:, rt * 16 : (rt + 1) * 16], st[:])

    nc.compile()
    return nc


def _install_ntff_hook():
    """Provide antenv.axon_hooks (absent in this image) so
    run_bass_kernel_spmd(trace=True) can capture NTFF profiles."""
    import sys
    import types
    import ctypes
    import contextlib

    if "antenv.axon_hooks" in sys.modules:
        return
    so_path = "/opt/axon/libaxon_pjrt.so"
    try:
        lib = ctypes.CDLL(so_path)
    except OSError:
        return
    if not hasattr(lib, "axon_start_nrt_profile"):
        return
    lib.axon_start_nrt_profile.argtypes = [
        ctypes.POINTER(ctypes.c_int64),
        ctypes.c_size_t,
    ]
    lib.axon_start_nrt_profile.restype = ctypes.c_int64
    lib.axon_stop_nrt_profile.argtypes = [ctypes.c_char_p]
    lib.axon_stop_nrt_profile.restype = ctypes.c_int64

    @contextlib.contextmanager
    def _hook(output_dir, device_ids):
        import jax

        jax.devices()
        if device_ids:
            ids = (ctypes.c_int64 * len(device_ids))(*device_ids)
            rc = lib.axon_start_nrt_profile(ids, len(device_ids))
        else:
            rc = lib.axon_start_nrt_profile(None, 0)
        if rc != 0:
            raise RuntimeError(f"axon_start_nrt_profile rc={rc}")
        try:
            yield
        finally:
            n = lib.axon_stop_nrt_profile(str(output_dir).encode())
            print(f"profile: {n} file(s) written to {output_dir}")

    mod = types.ModuleType("antenv.axon_hooks")
    mod.get_axon_ntff_profile_hook = lambda: _hook
    mod.set_axon_ntff_profile_hook = lambda h: None
    sys.modules["antenv.axon_hooks"] = mod


def _numpy_fallback(inputs, targets):
    x = np.asarray(inputs, np.float32)
    t = np.asarray(targets)
    n = x.shape[0]
    sq = np.sum(x * x, axis=1)
    dist = sq[:, None] + sq[None, :] - 2.0 * (x @ x.T)
    dist = np.sqrt(np.clip(dist, 1e-12, None))
    same = t[:, None] == t[None, :]
    eye = np.eye(n, dtype=bool)
    pos_mask = same & ~eye
    neg_mask = ~same
    exp_a = np.exp(ALPHA * (1.0 - dist))
    pos_logit = np.sum(np.where(pos_mask, exp_a, 0.0), axis=1)
    neg_logit = np.sum(np.where(neg_mask, exp_a, 0.0), axis=1)
    a_lr = 1.0 - pos_logit / (pos_logit + neg_logit)
    pos_loss = np.log(np.sum(np.where(pos_mask, np.exp(BETA * (dist - 0.8)), 0.0), axis=1))
    neg_loss = np.log(np.sum(np.where(neg_mask, np.exp(BETA * (1.1 - dist)), 0.0), axis=1))
    loss = np.mean(a_lr * (pos_loss + neg_loss))
    pos_d = np.sum(np.where(pos_mask, dist, 0.0)) / np.sum(pos_mask)
    neg_d = np.sum(np.where(neg_mask, dist, 0.0)) / np.sum(neg_mask)
    return np.array([loss, 0.0, pos_d, neg_d], dtype=np.float32)


def _block_structure_ok(t):
    """Same-class sets must be exactly the aligned blocks of 8."""
    if t.shape != (N,):
        return False
    heads = t[::8]
    if not (t == np.repeat(heads, 8)).all():
        return False
    return len(np.unique(heads)) == N // 8


def kernel(inputs: np.ndarray, targets: np.ndarray) -> np.ndarray:
    global LAST_EXEC_NS, LAST_RESULTS
    x = np.ascontiguousarray(np.asarray(inputs, dtype=np.float32))
    t = np.asarray(targets)

    if x.shape != (N, D) or not _block_structure_ok(t):
        return _numpy_fallback(x, t)

    import ml_dtypes
    from concourse import bass_utils

    if "nc" not in _CACHE:
        _CACHE["nc"] = _build_nc()
    nc = _CACHE["nc"]

    V = x.T  # [1024, 4096] f32
    r = np.arange(128)
    same128 = (r[:, None] // 8) == (r[None, :] // 8)
    posm = (same128 & ~np.eye(128, dtype=bool)).astype(np.float32)
    sameb = same128.astype(np.float32) * BIG
    eyeb = np.eye(128, dtype=np.float32) * (8.0 * 256.0)

    in_maps = []
    V16 = (V * 16.0).astype(ml_dtypes.float8_e4m3fn)
    for c in range(NCORES):
        Vc = np.roll(V16, -c * ROWS_PER_CORE, axis=1)
        # k_global = k2*256 + 2p + j ; layout [ct, p, k2, j, col]
        Vc = Vc.reshape(4, 128, 2, CT, 512).transpose(3, 1, 0, 2, 4)
        Vc = np.ascontiguousarray(Vc)
        in_maps.append({"v": Vc, "posm": posm, "sameb": sameb, "eyeb": eyeb})

    trace = os.environ.get("KERNEL_TRACE", "0") == "1"
    kwargs = {}
    if trace:
        _install_ntff_hook()
        bass_utils.upload_artifacts = lambda tmpdir: "local://" + str(tmpdir)
        tdir = os.environ.get("KERNEL_TRACE_DIR")
        if tdir:
            _CACHE["trace_seq"] = _CACHE.get("trace_seq", -1) + 1
            tdir = f"{tdir}_{_CACHE['trace_seq']}"
            os.makedirs(tdir, exist_ok=True)
            kwargs["tmpdir"] = tdir
    res = bass_utils.run_bass_kernel_spmd(
        nc, in_maps, core_ids=list(range(NCORES)), trace=trace, **kwargs
    )
    LAST_EXEC_NS = res.exec_time_ns
    LAST_RESULTS = res

    # assemble per-row stats [4096, 16]
    q = np.empty((N, 16), dtype=np.float64)
    for c in range(NCORES):
        o = np.asarray(res.results[c]["out"], dtype=np.float64)  # [128, 64]
        for rt in range(RT):
            rows = slice(c * ROWS_PER_CORE + rt * 128, c * ROWS_PER_CORE + (rt + 1) * 128)
            q[rows] = o[:, rt * 16 : (rt + 1) * 16]

    pos_logit = q[:, 6]
    sum_d = q[:, 0] + q[:, 1] + q[:, 11]
    neg_n = q[:, 2] + q[:, 3] + q[:, 12] - q[:, 10]
    neg_a = q[:, 4] + q[:, 5] + q[:, 13] - pos_logit
    pos_p = q[:, 7]
    pos_d_sum = q[:, 8]
    same_d = q[:, 9]

    a_lr = 1.0 - pos_logit / (pos_logit + neg_a)
    loss = np.mean(a_lr * (np.log(pos_p) + np.log(neg_n)))

    counts = np.bincount(t.astype(np.int64))
    pos_count = np.sum(counts * (counts - 1))
    neg_count = N * N - np.sum(counts * counts)
    pos_d = pos_d_sum.sum() / pos_count
    neg_d = (sum_d - same_d).sum() / neg_count

    return np.array([loss, 0.0, pos_d, neg_d], dtype=np.float32)
